# revision 13
# baseline (speedup 1.0000x reference)
"""Trainium2 Bass kernel for nn_AttentionMergeMask.

Reference computation (per sample b):
    K[c,k]   = (fg[c,k]+EPS) / ||fg[:,k]+EPS|| * m[k]        (k = pixel idx, 1024)
    att[c,p] = sum_k K[c,k] * A[k,p]                          (A = attention_scores[b])
    final    = att*(1-m) + fg*m
    out      = comb_w @ [fg; final] + comb_b

Strategy: pure data parallel, 4 samples per core on 8 cores. Per sample we
work in a transposed [pixel, channel] layout so the norm / mask operations
are per-partition scalars:
  - PE-transpose fg -> FT[pix, ch]; fused ACT Square+accum gives normsq[pix]
  - K^T = FT * (m * rsqrt(normsq))  (per-partition scale)
  - att^T[pixblk, c] = sum_kc matmul(lhsT=A[kc, pixblk], rhs=K^T[kc])
    (A is used in its natural layout as lhsT -- no transpose of A needed)
  - blend: final^T = FT*m + att^T*(1-m)  (one affine_then_add per block,
    with the (1-m) scale folded into the PSUM->SBUF evacuation on ACT)
  - PE-transpose final^T back to natural, then out = W^T-matmul over
    [fg; final] with comb_b added during PSUM evacuation.
EPS=1e-7 is dropped: its relative contribution is ~1e-7 (inputs are unit-scale
randn), far below fp32 matmul rounding.

Matmuls run as float32r (full PE rate for N>=256) by default; set
TRN_MM_F32R=0 for plain float32.
"""

import os
import numpy as np

NCORES = 8
BS, CH, H, W = 32, 256, 32, 32
HW = H * W                     # 1024
SPC = BS // NCORES             # samples per core = 4
NJ = HW // 128                 # 8 pixel chunks
NCB = CH // 128                # 2 channel blocks
NIC = (2 * CH) // 128          # 4 cat chunks

MM_F32R = os.environ.get("TRN_MM_F32R", "1") == "1"
T_F32R = os.environ.get("TRN_T_F32R", "0") == "1"
NEWTON = os.environ.get("TRN_NEWTON", "1") == "1"

_cache = {}


def _build():
    import concourse.bass as bass
    import concourse.tile as tile
    import concourse.mybir as mybir
    from concourse import bacc
    from concourse.bass import ts

    f32 = mybir.dt.float32
    f32r = mybir.dt.float32r
    fmm = f32r if MM_F32R else f32   # dtype for tensors feeding matmuls
    AF = mybir.ActivationFunctionType
    ALU = mybir.AluOpType

    nc = bacc.Bacc(
        "TRN2",
        target_bir_lowering=False,
        debug=False,
        enable_asserts=False,
    )
    fg_d = nc.dram_tensor("fg", [SPC, CH, HW], fmm, kind="ExternalInput")
    at_d = nc.dram_tensor("attn", [SPC, HW, HW], fmm, kind="ExternalInput")
    mt_d = nc.dram_tensor("mt", [SPC, 128, NJ], f32, kind="ExternalInput")
    wt_d = nc.dram_tensor("wt", [2 * CH, CH], fmm, kind="ExternalInput")
    b2_d = nc.dram_tensor("b2", [128, NCB], f32, kind="ExternalInput")
    id_d = nc.dram_tensor("ident", [128, 128], fmm, kind="ExternalInput")
    out_d = nc.dram_tensor("out", [SPC, CH, HW], f32, kind="ExternalOutput")

    def mm(ap):
        return ap

    def tt(ap):
        return ap

    with tile.TileContext(nc) as tc:
        with (
            tc.tile_pool(name="const", bufs=1) as cpool,
            tc.tile_pool(name="sb", bufs=2) as pool,
            tc.tile_pool(name="abuf", bufs=3) as apool,
            tc.tile_pool(name="psw", bufs=2, space=bass.MemorySpace.PSUM) as pswork,
            tc.tile_pool(name="psa", bufs=3, space=bass.MemorySpace.PSUM) as psatt,
            tc.tile_pool(name="ps2", bufs=2, space=bass.MemorySpace.PSUM) as psmm2,
        ):
            ident = cpool.tile([128, 128], fmm)
            nc.sync.dma_start(ident[:], id_d[:])
            wtt = cpool.tile([128, NIC, CH], fmm)
            nc.sync.dma_start(wtt[:], wt_d.rearrange("(ic p) o -> p ic o", p=128))
            b2t = cpool.tile([128, NCB], f32)
            nc.sync.dma_start(b2t[:], b2_d[:])

            for b in range(SPC):
                fgn = pool.tile([128, NCB, HW], fmm, tag="fgn")
                nc.sync.dma_start(fgn[:], fg_d[b].rearrange("(c p) f -> p c f", p=128))
                m_til = pool.tile([128, NJ], f32, tag="mt")
                nc.sync.dma_start(m_til[:], mt_d[b][:])

                ah = []
                for hh in range(2):
                    a = apool.tile([128, 4, HW], fmm, tag="A")
                    nc.sync.dma_start(
                        a[:],
                        at_d[b, hh * 512:(hh + 1) * 512, :].rearrange(
                            "(k p) f -> p k f", p=128
                        ),
                    )
                    ah.append(a)

                # ---- transpose fg -> FT[pix, ch], normsq via Square+accum ----
                ft = pool.tile([128, NJ, CH], f32, tag="ft")
                nsq = pool.tile([128, NJ], f32, tag="nsq")
                for j in range(NJ):
                    pst = pswork.tile([128, CH], fmm, tag="tw")
                    for ci in range(NCB):
                        nc.tensor.transpose(
                            pst[:, ci * 128:(ci + 1) * 128],
                            tt(fgn[:, ci, ts(j, 128)]),
                            tt(ident[:]),
                        )
                    scr = pool.tile([128, CH], f32, tag="scr")
                    nc.scalar.activation(
                        scr[:], pst[:], AF.Square, accum_out=nsq[:, j:j + 1]
                    )
                    nc.vector.tensor_copy(ft[:, j, :], pst[:])

                # ---- s = m * rsqrt(nsq), om = 1-m ----
                rin = pool.tile([128, NJ], f32, tag="rin")
                nc.vector.reciprocal(rin[:], nsq[:])
                rsq = pool.tile([128, NJ], f32, tag="rsq")
                nc.scalar.activation(rsq[:], rin[:], AF.Sqrt)
                if NEWTON:
                    t0 = pool.tile([128, NJ], f32, tag="nt0")
                    nc.vector.tensor_mul(t0[:], rsq[:], rsq[:])
                    nc.vector.tensor_mul(t0[:], t0[:], nsq[:])
                    nc.vector.tensor_scalar(
                        t0[:], t0[:], -0.5, 1.5, ALU.mult, ALU.add
                    )
                    nc.vector.tensor_mul(rsq[:], rsq[:], t0[:])
                s_til = pool.tile([128, NJ], f32, tag="stil")
                nc.vector.tensor_mul(s_til[:], rsq[:], m_til[:])
                om = pool.tile([128, NJ], f32, tag="om")
                nc.vector.tensor_scalar(om[:], m_til[:], -1.0, 1.0, ALU.mult, ALU.add)

                # ---- K^T = FT * s ----
                kt = pool.tile([128, NJ, CH], fmm, tag="kt")
                for j in range(NJ):
                    nc.vector.tensor_scalar_mul(
                        kt[:, j, :], ft[:, j, :], s_til[:, j:j + 1]
                    )

                # ---- mm1: att^T per pixel block; evac*(1-m); blend ----
                att_s = pool.tile([128, NJ, CH], f32, tag="atts")
                fin_t = pool.tile([128, NJ, CH], f32, tag="fint")
                for j in range(NJ):
                    psa = psatt.tile([128, CH], f32, tag="psa")
                    for kc in range(NJ):
                        nc.tensor.matmul(
                            psa[:],
                            mm(ah[kc // 4][:, kc % 4, ts(j, 128)]),
                            mm(kt[:, kc, :]),
                            start=(kc == 0),
                            stop=(kc == NJ - 1),
                        )
                    nc.scalar.activation(
                        att_s[:, j, :], psa[:], AF.Copy, bias=0.0,
                        scale=om[:, j:j + 1],
                    )
                    nc.vector.affine_then_add(
                        fin_t[:, j, :], ft[:, j, :], att_s[:, j, :],
                        scale=m_til[:, j:j + 1], bias=0.0,
                    )

                # ---- transpose final back to natural ----
                finaln = pool.tile([128, NCB, HW], fmm, tag="finaln")
                for ci in range(NCB):
                    for jg in range(2):
                        pso = pswork.tile([128, 512], f32, tag="tw")
                        for jj in range(4):
                            j = jg * 4 + jj
                            nc.tensor.transpose(
                                pso[:, jj * 128:(jj + 1) * 128],
                                fin_t[:, j, ts(ci, 128)],
                                ident[:].bitcast(f32),
                            )
                        nc.vector.tensor_copy(
                            finaln[:, ci, jg * 512:(jg + 1) * 512], pso[:]
                        )

                # ---- mm2: out = W^T @ [fg; final] + b ----
                outsb = pool.tile([128, NCB, HW], f32, tag="outsb")
                cats = [fgn[:, 0, :], fgn[:, 1, :], finaln[:, 0, :], finaln[:, 1, :]]
                for ob in range(NCB):
                    for nb in range(2):
                        ps2 = psmm2.tile([128, 512], f32, tag="ps2")
                        for ic in range(NIC):
                            nc.tensor.matmul(
                                ps2[:],
                                mm(wtt[:, ic, ts(ob, 128)]),
                                mm(cats[ic][:, ts(nb, 512)]),
                                start=(ic == 0),
                                stop=(ic == NIC - 1),
                            )
                        nc.vector.tensor_scalar_add(
                            outsb[:, ob, nb * 512:(nb + 1) * 512],
                            ps2[:],
                            b2t[:, ob:ob + 1],
                        )
                nc.sync.dma_start(
                    out_d[b].rearrange("(c p) f -> p c f", p=128), outsb[:]
                )
    nc.compile()
    return nc


def _prep_inputs(foreground, mask, attention_scores, comb_w, comb_b):
    f32 = np.float32
    fg = np.ascontiguousarray(foreground.reshape(BS, CH, HW), dtype=f32)
    at = np.ascontiguousarray(attention_scores.reshape(BS, HW, HW), dtype=f32)
    m = np.asarray(mask, dtype=f32).reshape(BS, HW)
    mt = np.ascontiguousarray(m.reshape(BS, NJ, 128).transpose(0, 2, 1))
    wt = np.ascontiguousarray(np.asarray(comb_w, dtype=f32).T)
    b2 = np.ascontiguousarray(np.asarray(comb_b, dtype=f32).reshape(NCB, 128).T)
    ident = np.eye(128, dtype=f32)
    in_maps = []
    for c in range(NCORES):
        sl = slice(c * SPC, (c + 1) * SPC)
        in_maps.append({
            "fg": np.ascontiguousarray(fg[sl]),
            "attn": np.ascontiguousarray(at[sl]),
            "mt": np.ascontiguousarray(mt[sl]),
            "wt": wt,
            "b2": b2,
            "ident": ident,
        })
    return in_maps


def run(inputs, trace=False):
    from concourse.bass_utils import run_bass_kernel_spmd

    if "nc" not in _cache:
        _cache["nc"] = _build()
    nc = _cache["nc"]
    in_maps = _prep_inputs(**inputs)
    res = run_bass_kernel_spmd(
        nc, in_maps, core_ids=list(range(NCORES)), trace=trace,
    )
    outs = [r["out"] for r in res.results]
    full = np.concatenate(outs, axis=0).reshape(BS, CH, H, W)
    return full, res


def kernel(**inputs) -> np.ndarray:
    out, _ = run(inputs, trace=False)
    return out


def bench(inputs, iters=20):
    """Build the sharded PJRT executable once, keep inputs device-resident,
    and time repeated executions (min over iters). Mirrors
    bass2jax.run_bass_via_pjrt's multi-core path without per-call retracing."""
    import time

    import jax
    import concourse.mybir as mybir
    from concourse.bass2jax import (
        _bass_exec_p,
        install_neuronx_cc_hook,
        partition_id_tensor,
        Mesh,
        PartitionSpec,
        shard_map,
    )

    install_neuronx_cc_hook()
    if "nc" not in _cache:
        _cache["nc"] = _build()
    nc = _cache["nc"]
    in_maps = _prep_inputs(**inputs)

    partition_name = (
        nc.partition_id_tensor.name if nc.partition_id_tensor else None
    )
    in_names, out_names, out_avals, zero_outs = [], [], [], []
    for alloc in nc.m.functions[0].allocations:
        if not isinstance(alloc, mybir.MemoryLocationSet):
            continue
        name = alloc.memorylocations[0].name
        if alloc.kind == "ExternalInput":
            if name != partition_name:
                in_names.append(name)
        elif alloc.kind == "ExternalOutput":
            out_names.append(name)
            shape = tuple(alloc.tensor_shape)
            dtype = mybir.dt.np(alloc.dtype)
            out_avals.append(jax.core.ShapedArray(shape, dtype))
            zero_outs.append(np.zeros(shape, dtype))
    n_params = len(in_names)
    all_in_names = in_names + out_names
    if partition_name is not None:
        all_in_names = all_in_names + [partition_name]

    def _body(*args):
        operands = list(args)
        if partition_name is not None:
            operands.append(partition_id_tensor())
        outs = _bass_exec_p.bind(
            *operands,
            out_avals=tuple(out_avals),
            in_names=tuple(all_in_names),
            out_names=tuple(out_names),
            lowering_input_output_aliases=(),
            sim_require_finite=True,
            sim_require_nnan=True,
            nc=nc,
        )
        return tuple(outs)

    devices = jax.devices()[:NCORES]
    mesh = Mesh(np.asarray(devices), ("core",))
    in_specs = (PartitionSpec("core"),) * (n_params + len(out_names))
    out_specs = (PartitionSpec("core"),) * len(out_names)
    sharded = jax.jit(
        shard_map(_body, mesh=mesh, in_specs=in_specs, out_specs=out_specs,
                  check_rep=False),
        keep_unused=True,
    )
    concat_in = [
        np.concatenate([in_maps[c][nm] for c in range(NCORES)], axis=0)
        for nm in in_names
    ]
    concat_zeros = [
        np.zeros((NCORES * z.shape[0], *z.shape[1:]), z.dtype) for z in zero_outs
    ]
    sharding = jax.sharding.NamedSharding(mesh, PartitionSpec("core"))
    dev_in = [jax.device_put(x, sharding) for x in concat_in]
    dev_zero = [jax.device_put(x, sharding) for x in concat_zeros]

    # warmup (compiles)
    out = sharded(*dev_in, *dev_zero)
    jax.block_until_ready(out)
    times = []
    for _ in range(iters):
        t0 = time.perf_counter()
        out = sharded(*dev_in, *dev_zero)
        jax.block_until_ready(out)
        times.append(time.perf_counter() - t0)
    full = (
        np.asarray(out[0])
        .reshape(NCORES * SPC, CH, HW)
        .reshape(BS, CH, H, W)
    )
    return full, times


def bench_chain(inputs, n_chain=64, iters=8):
    """Time N chained NEFF executions inside one dispatch; the slope
    (T_chain - T_single)/(n_chain-1) removes the ~40-80ms axon RPC overhead."""
    import time

    import jax
    import concourse.mybir as mybir
    from concourse.bass2jax import (
        _bass_exec_p,
        install_neuronx_cc_hook,
        partition_id_tensor,
        Mesh,
        PartitionSpec,
        shard_map,
    )

    install_neuronx_cc_hook()
    if "nc" not in _cache:
        _cache["nc"] = _build()
    nc = _cache["nc"]
    in_maps = _prep_inputs(**inputs)

    partition_name = (
        nc.partition_id_tensor.name if nc.partition_id_tensor else None
    )
    in_names, out_names, out_avals, zero_outs = [], [], [], []
    for alloc in nc.m.functions[0].allocations:
        if not isinstance(alloc, mybir.MemoryLocationSet):
            continue
        name = alloc.memorylocations[0].name
        if alloc.kind == "ExternalInput":
            if name != partition_name:
                in_names.append(name)
        elif alloc.kind == "ExternalOutput":
            out_names.append(name)
            shape = tuple(alloc.tensor_shape)
            dtype = mybir.dt.np(alloc.dtype)
            out_avals.append(jax.core.ShapedArray(shape, dtype))
            zero_outs.append(np.zeros(shape, dtype))
    n_params = len(in_names)
    all_in_names = in_names + out_names
    if partition_name is not None:
        all_in_names = all_in_names + [partition_name]

    def _body(*args):
        operands = list(args)
        if partition_name is not None:
            operands.append(partition_id_tensor())
        return tuple(_bass_exec_p.bind(
            *operands,
            out_avals=tuple(out_avals),
            in_names=tuple(all_in_names),
            out_names=tuple(out_names),
            lowering_input_output_aliases=(),
            sim_require_finite=True,
            sim_require_nnan=True,
            nc=nc,
        ))

    devices = jax.devices()[:NCORES]
    mesh = Mesh(np.asarray(devices), ("core",))
    in_specs = (PartitionSpec("core"),) * (n_params + len(out_names))
    out_specs = (PartitionSpec("core"),) * len(out_names)
    sharded = jax.jit(
        shard_map(_body, mesh=mesh, in_specs=in_specs,
                  out_specs=out_specs, check_rep=False),
        keep_unused=True,
    )

    concat_in = [
        np.concatenate([in_maps[c][nm] for c in range(NCORES)], axis=0)
        for nm in in_names
    ]
    concat_zeros = [
        np.zeros((NCORES * z.shape[0], *z.shape[1:]), z.dtype) for z in zero_outs
    ]
    sharding = jax.sharding.NamedSharding(mesh, PartitionSpec("core"))
    dev_in = [jax.device_put(x, sharding) for x in concat_in]
    dev_zero = [jax.device_put(x, sharding) for x in concat_zeros]

    def run_n(n):
        # async-dispatch n executions, chained through the donated output
        # buffers so they serialize on-device; block once at the end
        outs = dev_zero
        for _ in range(n):
            outs = list(sharded(*dev_in, *outs))
        jax.block_until_ready(outs)
        return outs

    out = run_n(1)  # warmup / compile

    def timed(n):
        best = 1e18
        for _ in range(iters):
            t0 = time.perf_counter()
            run_n(n)
            best = min(best, time.perf_counter() - t0)
        return best

    n1 = 2
    t1 = timed(n1)
    tn = timed(n_chain)
    out = run_n(1)
    per_exec = (tn - t1) / (n_chain - n1)
    full = (
        np.asarray(out[0])
        .reshape(NCORES * SPC, CH, HW)
        .reshape(BS, CH, H, W)
    )
    return full, per_exec, (t1, tn)


# revision 14
# speedup vs baseline: 1.0920x; 1.0920x over previous
"""Trainium2 Bass kernel for nn_AttentionMergeMask.

Reference computation (per sample b):
    K[c,k]   = (fg[c,k]+EPS) / ||fg[:,k]+EPS|| * m[k]        (k = pixel idx, 1024)
    att[c,p] = sum_k K[c,k] * A[k,p]                          (A = attention_scores[b])
    final    = att*(1-m) + fg*m
    out      = comb_w @ [fg; final] + comb_b

Strategy: pure data parallel, 4 samples per core on 8 cores. Per sample we
work in a transposed [pixel, channel] layout so the norm / mask operations
are per-partition scalars:
  - PE-transpose fg -> FT[pix, ch]; fused ACT Square+accum gives normsq[pix]
  - K^T = FT * (m * rsqrt(normsq))  (per-partition scale)
  - att^T[pixblk, c] = sum_kc matmul(lhsT=A[kc, pixblk], rhs=K^T[kc])
    (A is used in its natural layout as lhsT -- no transpose of A needed)
  - blend: final^T = FT*m + att^T*(1-m)  (one affine_then_add per block,
    with the (1-m) scale folded into the PSUM->SBUF evacuation on ACT)
  - PE-transpose final^T back to natural, then out = W^T-matmul over
    [fg; final] with comb_b added during PSUM evacuation.
EPS=1e-7 is dropped: its relative contribution is ~1e-7 (inputs are unit-scale
randn), far below fp32 matmul rounding.

Matmuls run as float32r (full PE rate for N>=256) by default; set
TRN_MM_F32R=0 for plain float32.
"""

import os
import numpy as np

NCORES = 8
BS, CH, H, W = 32, 256, 32, 32
HW = H * W                     # 1024
SPC = BS // NCORES             # samples per core = 4
NJ = HW // 128                 # 8 pixel chunks
NCB = CH // 128                # 2 channel blocks
NIC = (2 * CH) // 128          # 4 cat chunks

MM_F32R = os.environ.get("TRN_MM_F32R", "1") == "1"      # mm1 (A @ K^T)
MM2_F32R = os.environ.get("TRN_MM2_F32R", "0") == "1"    # mm2 (W @ cat)
TIN_F32R = os.environ.get("TRN_TIN_F32R", "0") == "1"    # fg transposes
T_F32R = os.environ.get("TRN_T_F32R", "0") == "1"
NEWTON = os.environ.get("TRN_NEWTON", "1") == "1"

_cache = {}


def _build():
    import concourse.bass as bass
    import concourse.tile as tile
    import concourse.mybir as mybir
    from concourse import bacc
    from concourse.bass import ts

    f32 = mybir.dt.float32
    f32r = mybir.dt.float32r
    d_mm1 = f32r if MM_F32R else f32    # ah, kt
    d_mm2 = f32r if MM2_F32R else f32   # wtt, fgn(mm2 rhs), finaln
    d_tin = f32r if TIN_F32R else f32   # fgn/ident/pst transpose path
    AF = mybir.ActivationFunctionType
    ALU = mybir.AluOpType

    nc = bacc.Bacc(
        "TRN2",
        target_bir_lowering=False,
        debug=False,
        enable_asserts=False,
    )
    fg_d = nc.dram_tensor("fg", [SPC, CH, HW], d_tin, kind="ExternalInput")
    at_d = nc.dram_tensor("attn", [SPC, HW, HW], d_mm1, kind="ExternalInput")
    mt_d = nc.dram_tensor("mt", [SPC, 128, NJ], f32, kind="ExternalInput")
    wt_d = nc.dram_tensor("wt", [2 * CH, CH], d_mm2, kind="ExternalInput")
    b2_d = nc.dram_tensor("b2", [128, NCB], f32, kind="ExternalInput")
    id_d = nc.dram_tensor("ident", [128, 128], d_tin, kind="ExternalInput")
    out_d = nc.dram_tensor("out", [SPC, CH, HW], f32, kind="ExternalOutput")

    def mm(ap):
        return ap

    def tt(ap):
        return ap

    with tile.TileContext(nc) as tc:
        with (
            tc.tile_pool(name="const", bufs=1) as cpool,
            tc.tile_pool(name="sb", bufs=2) as pool,
            tc.tile_pool(name="abuf", bufs=3) as apool,
            tc.tile_pool(name="psw", bufs=2, space=bass.MemorySpace.PSUM) as pswork,
            tc.tile_pool(name="psa", bufs=3, space=bass.MemorySpace.PSUM) as psatt,
            tc.tile_pool(name="ps2", bufs=2, space=bass.MemorySpace.PSUM) as psmm2,
        ):
            ident = cpool.tile([128, 128], d_tin)
            nc.sync.dma_start(ident[:], id_d[:])
            wtt = cpool.tile([128, NIC, CH], d_mm2)
            nc.sync.dma_start(wtt[:], wt_d.rearrange("(ic p) o -> p ic o", p=128))
            b2t = cpool.tile([128, NCB], f32)
            nc.sync.dma_start(b2t[:], b2_d[:])

            for b in range(SPC):
                fgn = pool.tile([128, NCB, HW], d_tin, tag="fgn")
                nc.sync.dma_start(fgn[:], fg_d[b].rearrange("(c p) f -> p c f", p=128))
                m_til = pool.tile([128, NJ], f32, tag="mt")
                nc.sync.dma_start(m_til[:], mt_d[b][:])

                ah = []
                for hh in range(2):
                    a = apool.tile([128, 4, HW], d_mm1, tag="A")
                    nc.sync.dma_start(
                        a[:],
                        at_d[b, hh * 512:(hh + 1) * 512, :].rearrange(
                            "(k p) f -> p k f", p=128
                        ),
                    )
                    ah.append(a)

                # ---- transpose fg -> FT[pix, ch], normsq via Square+accum ----
                ft = pool.tile([128, NJ, CH], f32, tag="ft")
                nsq = pool.tile([128, NJ], f32, tag="nsq")
                for j in range(NJ):
                    pst = pswork.tile([128, CH], d_tin, tag="tw")
                    for ci in range(NCB):
                        nc.tensor.transpose(
                            pst[:, ci * 128:(ci + 1) * 128],
                            tt(fgn[:, ci, ts(j, 128)]),
                            tt(ident[:]),
                        )
                    scr = pool.tile([128, CH], f32, tag="scr")
                    nc.scalar.activation(
                        scr[:], pst[:], AF.Square, accum_out=nsq[:, j:j + 1]
                    )
                    nc.vector.tensor_copy(ft[:, j, :], pst[:])

                # ---- s = m * rsqrt(nsq), om = 1-m ----
                rin = pool.tile([128, NJ], f32, tag="rin")
                nc.vector.reciprocal(rin[:], nsq[:])
                rsq = pool.tile([128, NJ], f32, tag="rsq")
                nc.scalar.activation(rsq[:], rin[:], AF.Sqrt)
                if NEWTON:
                    t0 = pool.tile([128, NJ], f32, tag="nt0")
                    nc.vector.tensor_mul(t0[:], rsq[:], rsq[:])
                    nc.vector.tensor_mul(t0[:], t0[:], nsq[:])
                    nc.vector.tensor_scalar(
                        t0[:], t0[:], -0.5, 1.5, ALU.mult, ALU.add
                    )
                    nc.vector.tensor_mul(rsq[:], rsq[:], t0[:])
                s_til = pool.tile([128, NJ], f32, tag="stil")
                nc.vector.tensor_mul(s_til[:], rsq[:], m_til[:])
                om = pool.tile([128, NJ], f32, tag="om")
                nc.vector.tensor_scalar(om[:], m_til[:], -1.0, 1.0, ALU.mult, ALU.add)

                # ---- K^T = FT * s ----
                kt = pool.tile([128, NJ, CH], d_mm1, tag="kt")
                for j in range(NJ):
                    nc.vector.tensor_scalar_mul(
                        kt[:, j, :], ft[:, j, :], s_til[:, j:j + 1]
                    )

                # ---- mm1: att^T per pixel block; evac*(1-m); blend ----
                att_s = pool.tile([128, NJ, CH], f32, tag="atts")
                fin_t = pool.tile([128, NJ, CH], f32, tag="fint")
                for j in range(NJ):
                    psa = psatt.tile([128, CH], f32, tag="psa")
                    for kc in range(NJ):
                        nc.tensor.matmul(
                            psa[:],
                            mm(ah[kc // 4][:, kc % 4, ts(j, 128)]),
                            mm(kt[:, kc, :]),
                            start=(kc == 0),
                            stop=(kc == NJ - 1),
                        )
                    nc.scalar.activation(
                        att_s[:, j, :], psa[:], AF.Copy, bias=0.0,
                        scale=om[:, j:j + 1],
                    )
                    nc.vector.affine_then_add(
                        fin_t[:, j, :], ft[:, j, :], att_s[:, j, :],
                        scale=m_til[:, j:j + 1], bias=0.0,
                    )

                # ---- transpose final back to natural ----
                finaln = pool.tile([128, NCB, HW], d_mm2, tag="finaln")
                for ci in range(NCB):
                    for jg in range(2):
                        pso = pswork.tile([128, 512], f32, tag="tw")
                        for jj in range(4):
                            j = jg * 4 + jj
                            nc.tensor.transpose(
                                pso[:, jj * 128:(jj + 1) * 128],
                                fin_t[:, j, ts(ci, 128)],
                                ident[:].bitcast(f32),
                            )
                        nc.vector.tensor_copy(
                            finaln[:, ci, jg * 512:(jg + 1) * 512], pso[:]
                        )

                # ---- mm2: out = W^T @ [fg; final] + b ----
                outsb = pool.tile([128, NCB, HW], f32, tag="outsb")
                if d_mm2 != d_tin:
                    fgn2 = pool.tile([128, NCB, HW], d_mm2, tag="fgn2")
                    for ci in range(NCB):
                        nc.gpsimd.tensor_copy(fgn2[:, ci, :], fgn[:, ci, :])
                else:
                    fgn2 = fgn
                cats = [fgn2[:, 0, :], fgn2[:, 1, :], finaln[:, 0, :], finaln[:, 1, :]]
                for ob in range(NCB):
                    for nb in range(2):
                        ps2 = psmm2.tile([128, 512], f32, tag="ps2")
                        for ic in range(NIC):
                            nc.tensor.matmul(
                                ps2[:],
                                mm(wtt[:, ic, ts(ob, 128)]),
                                mm(cats[ic][:, ts(nb, 512)]),
                                start=(ic == 0),
                                stop=(ic == NIC - 1),
                            )
                        nc.vector.tensor_scalar_add(
                            outsb[:, ob, nb * 512:(nb + 1) * 512],
                            ps2[:],
                            b2t[:, ob:ob + 1],
                        )
                nc.sync.dma_start(
                    out_d[b].rearrange("(c p) f -> p c f", p=128), outsb[:]
                )
    nc.compile()
    return nc


def _prep_inputs(foreground, mask, attention_scores, comb_w, comb_b):
    f32 = np.float32
    fg = np.ascontiguousarray(foreground.reshape(BS, CH, HW), dtype=f32)
    at = np.ascontiguousarray(attention_scores.reshape(BS, HW, HW), dtype=f32)
    m = np.asarray(mask, dtype=f32).reshape(BS, HW)
    mt = np.ascontiguousarray(m.reshape(BS, NJ, 128).transpose(0, 2, 1))
    wt = np.ascontiguousarray(np.asarray(comb_w, dtype=f32).T)
    b2 = np.ascontiguousarray(np.asarray(comb_b, dtype=f32).reshape(NCB, 128).T)
    ident = np.eye(128, dtype=f32)
    in_maps = []
    for c in range(NCORES):
        sl = slice(c * SPC, (c + 1) * SPC)
        in_maps.append({
            "fg": np.ascontiguousarray(fg[sl]),
            "attn": np.ascontiguousarray(at[sl]),
            "mt": np.ascontiguousarray(mt[sl]),
            "wt": wt,
            "b2": b2,
            "ident": ident,
        })
    return in_maps


def run(inputs, trace=False):
    from concourse.bass_utils import run_bass_kernel_spmd

    if "nc" not in _cache:
        _cache["nc"] = _build()
    nc = _cache["nc"]
    in_maps = _prep_inputs(**inputs)
    res = run_bass_kernel_spmd(
        nc, in_maps, core_ids=list(range(NCORES)), trace=trace,
    )
    outs = [r["out"] for r in res.results]
    full = np.concatenate(outs, axis=0).reshape(BS, CH, H, W)
    return full, res


def kernel(**inputs) -> np.ndarray:
    out, _ = run(inputs, trace=False)
    return out


def bench(inputs, iters=20):
    """Build the sharded PJRT executable once, keep inputs device-resident,
    and time repeated executions (min over iters). Mirrors
    bass2jax.run_bass_via_pjrt's multi-core path without per-call retracing."""
    import time

    import jax
    import concourse.mybir as mybir
    from concourse.bass2jax import (
        _bass_exec_p,
        install_neuronx_cc_hook,
        partition_id_tensor,
        Mesh,
        PartitionSpec,
        shard_map,
    )

    install_neuronx_cc_hook()
    if "nc" not in _cache:
        _cache["nc"] = _build()
    nc = _cache["nc"]
    in_maps = _prep_inputs(**inputs)

    partition_name = (
        nc.partition_id_tensor.name if nc.partition_id_tensor else None
    )
    in_names, out_names, out_avals, zero_outs = [], [], [], []
    for alloc in nc.m.functions[0].allocations:
        if not isinstance(alloc, mybir.MemoryLocationSet):
            continue
        name = alloc.memorylocations[0].name
        if alloc.kind == "ExternalInput":
            if name != partition_name:
                in_names.append(name)
        elif alloc.kind == "ExternalOutput":
            out_names.append(name)
            shape = tuple(alloc.tensor_shape)
            dtype = mybir.dt.np(alloc.dtype)
            out_avals.append(jax.core.ShapedArray(shape, dtype))
            zero_outs.append(np.zeros(shape, dtype))
    n_params = len(in_names)
    all_in_names = in_names + out_names
    if partition_name is not None:
        all_in_names = all_in_names + [partition_name]

    def _body(*args):
        operands = list(args)
        if partition_name is not None:
            operands.append(partition_id_tensor())
        outs = _bass_exec_p.bind(
            *operands,
            out_avals=tuple(out_avals),
            in_names=tuple(all_in_names),
            out_names=tuple(out_names),
            lowering_input_output_aliases=(),
            sim_require_finite=True,
            sim_require_nnan=True,
            nc=nc,
        )
        return tuple(outs)

    devices = jax.devices()[:NCORES]
    mesh = Mesh(np.asarray(devices), ("core",))
    in_specs = (PartitionSpec("core"),) * (n_params + len(out_names))
    out_specs = (PartitionSpec("core"),) * len(out_names)
    sharded = jax.jit(
        shard_map(_body, mesh=mesh, in_specs=in_specs, out_specs=out_specs,
                  check_rep=False),
        keep_unused=True,
    )
    concat_in = [
        np.concatenate([in_maps[c][nm] for c in range(NCORES)], axis=0)
        for nm in in_names
    ]
    concat_zeros = [
        np.zeros((NCORES * z.shape[0], *z.shape[1:]), z.dtype) for z in zero_outs
    ]
    sharding = jax.sharding.NamedSharding(mesh, PartitionSpec("core"))
    dev_in = [jax.device_put(x, sharding) for x in concat_in]
    dev_zero = [jax.device_put(x, sharding) for x in concat_zeros]

    # warmup (compiles)
    out = sharded(*dev_in, *dev_zero)
    jax.block_until_ready(out)
    times = []
    for _ in range(iters):
        t0 = time.perf_counter()
        out = sharded(*dev_in, *dev_zero)
        jax.block_until_ready(out)
        times.append(time.perf_counter() - t0)
    full = (
        np.asarray(out[0])
        .reshape(NCORES * SPC, CH, HW)
        .reshape(BS, CH, H, W)
    )
    return full, times


def bench_chain(inputs, n_chain=64, iters=8):
    """Time N chained NEFF executions inside one dispatch; the slope
    (T_chain - T_single)/(n_chain-1) removes the ~40-80ms axon RPC overhead."""
    import time

    import jax
    import concourse.mybir as mybir
    from concourse.bass2jax import (
        _bass_exec_p,
        install_neuronx_cc_hook,
        partition_id_tensor,
        Mesh,
        PartitionSpec,
        shard_map,
    )

    install_neuronx_cc_hook()
    if "nc" not in _cache:
        _cache["nc"] = _build()
    nc = _cache["nc"]
    in_maps = _prep_inputs(**inputs)

    partition_name = (
        nc.partition_id_tensor.name if nc.partition_id_tensor else None
    )
    in_names, out_names, out_avals, zero_outs = [], [], [], []
    for alloc in nc.m.functions[0].allocations:
        if not isinstance(alloc, mybir.MemoryLocationSet):
            continue
        name = alloc.memorylocations[0].name
        if alloc.kind == "ExternalInput":
            if name != partition_name:
                in_names.append(name)
        elif alloc.kind == "ExternalOutput":
            out_names.append(name)
            shape = tuple(alloc.tensor_shape)
            dtype = mybir.dt.np(alloc.dtype)
            out_avals.append(jax.core.ShapedArray(shape, dtype))
            zero_outs.append(np.zeros(shape, dtype))
    n_params = len(in_names)
    all_in_names = in_names + out_names
    if partition_name is not None:
        all_in_names = all_in_names + [partition_name]

    def _body(*args):
        operands = list(args)
        if partition_name is not None:
            operands.append(partition_id_tensor())
        return tuple(_bass_exec_p.bind(
            *operands,
            out_avals=tuple(out_avals),
            in_names=tuple(all_in_names),
            out_names=tuple(out_names),
            lowering_input_output_aliases=(),
            sim_require_finite=True,
            sim_require_nnan=True,
            nc=nc,
        ))

    devices = jax.devices()[:NCORES]
    mesh = Mesh(np.asarray(devices), ("core",))
    in_specs = (PartitionSpec("core"),) * (n_params + len(out_names))
    out_specs = (PartitionSpec("core"),) * len(out_names)
    sharded = jax.jit(
        shard_map(_body, mesh=mesh, in_specs=in_specs,
                  out_specs=out_specs, check_rep=False),
        keep_unused=True,
    )

    concat_in = [
        np.concatenate([in_maps[c][nm] for c in range(NCORES)], axis=0)
        for nm in in_names
    ]
    concat_zeros = [
        np.zeros((NCORES * z.shape[0], *z.shape[1:]), z.dtype) for z in zero_outs
    ]
    sharding = jax.sharding.NamedSharding(mesh, PartitionSpec("core"))
    dev_in = [jax.device_put(x, sharding) for x in concat_in]
    dev_zero = [jax.device_put(x, sharding) for x in concat_zeros]

    def run_n(n):
        # async-dispatch n executions, chained through the donated output
        # buffers so they serialize on-device; block once at the end
        outs = dev_zero
        for _ in range(n):
            outs = list(sharded(*dev_in, *outs))
        jax.block_until_ready(outs)
        return outs

    out = run_n(1)  # warmup / compile

    def timed(n):
        best = 1e18
        for _ in range(iters):
            t0 = time.perf_counter()
            run_n(n)
            best = min(best, time.perf_counter() - t0)
        return best

    n1 = 2
    t1 = timed(n1)
    tn = timed(n_chain)
    out = run_n(1)
    per_exec = (tn - t1) / (n_chain - n1)
    full = (
        np.asarray(out[0])
        .reshape(NCORES * SPC, CH, HW)
        .reshape(BS, CH, H, W)
    )
    return full, per_exec, (t1, tn)
